# revision 4
# baseline (speedup 1.0000x reference)
"""Trainium2 Bass kernel for PixelUnshuffle->MHA->PixelShuffle (nn_Attention).

Reference computation (per batch element, 8 batch elements data-parallel
across 8 NeuronCores):
  x [64, 256, 256] --PixelUnshuffle(8)--> tokens [N=1024, C=4096]
  qkv = tokens @ W_qkv            [1024, 768]
  4-head attention (d=64), softmax over tokens
  y = attn_out @ W_out + b_out    [1024, 4096]
  --PixelShuffle(8)--> [64, 256, 256]

Key layout trick: the pixel un/shuffle is folded into the matmuls.
x is DMA'd in its natural (fully contiguous) layout, then a single
cast+de-stride engine copy produces bf16 tiles in (r2, hh, ww) order so
every matmul operand is contiguous. The QKV projection iterates over the
inner shuffle coordinate r2 (8 values); the output projection writes its
PSUM results back through strided evacuation copies straight into the
pixel-shuffled output layout. All DMA transfers move >=1KB contiguous
runs. Weights are host-side pre-permuted and pre-cast to bf16.

Token index   n = hh*32 + ww            (hh, ww in [0,32))
Channel index c = c0*64 + r1*8 + r2     (c0 in [0,64), r1, r2 in [0,8))
x[c0, hh*8+r1, ww*8+r2] = tokens[n, c]

Attention is computed transposed (dotsT[m, n] with the summed token m on
partitions) so that no on-chip transposes are needed anywhere:
  dotsT[m,n] = kT_h[:, m-chunk] (lhsT) x qT_h (rhs)  -> exp -> edotsT
  out_augT[i', n] = sum_m v_aug[m, i'] * edotsT[m, n]  with v_aug
  carrying an extra ones column so row 64 accumulates the softmax
  denominator Z[n] for free. 1/Z is computed on 64 lanes (an SBUF->SBUF
  DMA redistributes [1,1024] -> [64,16] before the reciprocal) and
  broadcast across partitions with a 0-stride DRAM load (gpsimd
  partition_broadcast gives wrong results on HW for non-zero base
  partitions; 0-stride partition APs are only legal for DRAM sources).
"""

import sys

if "/opt/trn_rl_repo" not in sys.path:
    sys.path.insert(0, "/opt/trn_rl_repo")

import os

import ml_dtypes
import numpy as np

import concourse.bass as bass
from concourse import bacc, mybir, tile
from concourse.bass_utils import run_bass_kernel_spmd

F32 = mybir.dt.float32
BF16 = mybir.dt.bfloat16

SCALE = 0.125  # DIM_HEAD ** -0.5

_CACHE = {}


def _build(debug_outs=False, zero_bias=False):
    nc = bacc.Bacc("TRN2", target_bir_lowering=False, debug=False, num_devices=8)

    x_d = nc.dram_tensor("x", [64, 256, 256], BF16, kind="ExternalInput").ap()
    wq_d = nc.dram_tensor("W_qkv", [4096, 768], BF16, kind="ExternalInput").ap()
    wo_d = nc.dram_tensor("W_out", [256, 4096], BF16, kind="ExternalInput").ap()
    b_d = nc.dram_tensor("b_out", [4096], F32, kind="ExternalInput").ap()
    out_d = nc.dram_tensor("out", [64, 256, 256], BF16, kind="ExternalOutput").ap()

    zsc_d = nc.dram_tensor("z_scratch", [4, 1024], F32).ap()
    zrc_d = nc.dram_tensor("zr_scratch", [4, 1024], F32).ap()

    dbg = None
    if debug_outs:
        dbg = {
            "qkT": nc.dram_tensor(
                "dbg_qkT", [128, 4, 1024], F32, kind="ExternalOutput"
            ).ap(),
            "v_sb": nc.dram_tensor(
                "dbg_v", [128, 8, 4, 68], F32, kind="ExternalOutput"
            ).ap(),
            "outT": nc.dram_tensor(
                "dbg_outT", [128, 2, 1024], F32, kind="ExternalOutput"
            ).ap(),
            "zbc": nc.dram_tensor(
                "dbg_zbc", [64, 4, 1024], F32, kind="ExternalOutput"
            ).ap(),
        }

    def dram_ap(base, off, pattern):
        return bass.AP(tensor=base.tensor, offset=base.offset + off, ap=pattern)

    with tile.TileContext(nc) as tc:
        _build_tiled(nc, tc, x_d, wq_d, wo_d, b_d, out_d, zsc_d, zrc_d, dram_ap, dbg, zero_bias)
    nc.compile()
    return nc


def _build_tiled(nc, tc, x_d, wq_d, wo_d, b_d, out_d, zsc_d, zrc_d, dram_ap, dbg=None, zero_bias=False):
    from contextlib import ExitStack

    with ExitStack() as ctx:
        pers = ctx.enter_context(tc.tile_pool(name="pers", bufs=1))
        s23 = ctx.enter_context(tc.tile_pool(name="s23", bufs=1))

        # ---- persistent tiles ----
        # qkT[d-part, ot, n] : ot 0,1 = q dims 0..128,128..256; ot 2,3 = k
        qkT = pers.tile([128, 4, 1024], BF16)
        # v_aug[m-part, mc, h, 68] bf16, col 64 = ones (65-67 pad for align)
        v_sb = pers.tile([128, 8, 4, 68], BF16)
        # outT[i-part, ic, n] : i = h*64+d ; ic = i//128
        outT = pers.tile([128, 2, 1024], BF16)
        # bias[c-part, r2, cg]
        bias_sb = pers.tile([128, 8, 4], F32)
        # W_out tile in outer pool; DMA issued behind stage-1 critical loads
        wo_sb = s23.tile([128, 2, 4096], BF16)  # [i-part, ic, c_perm]

        nc.vector.memset(v_sb[:, :, :, 64:68], 1.0)
        # host pre-arranges b_out as [p, r2, cg] so this is a flat copy
        nc.gpsimd.dma_start(
            out=bias_sb[:],
            in_=dram_ap(b_d, 0, [[32, 128], [4, 8], [1, 4]]),
        )

        # =========================== stage 1 ===========================
        # QKV projection with pixel-unshuffle folded in. 2 windows of 512
        # tokens (hh-halves); x staged per half-window (8 hh rows).
        with (
            tc.tile_pool(name="wq", bufs=1) as wqp,
            tc.tile_pool(name="xw", bufs=1) as xwp,
            tc.tile_pool(name="ps1", bufs=1, space="PSUM") as ps1,
        ):
            wq_sb = wqp.tile([128, 8, 4, 768], BF16)  # [c-part, r2, cg, o]

            # PE warmup: ~5us of dummy matmuls so HAM reaches 2.4 GHz
            # before the first real matmul arrives
            warm = wqp.tile([128, 512], BF16)
            nc.vector.memset(warm[:], 0.0)
            warm_ps = ps1.tile([128, 512], F32, tag="qk0", bufs=1)
            for i in range(24):
                nc.tensor.matmul(
                    warm_ps[:], warm[:, 0:128], warm[:], start=True, stop=True
                )

            def load_wq(r2, engs):
                # two half-chunks (cg pairs) so no single transfer hogs a lane
                for i, eng in enumerate(engs):
                    eng.dma_start(
                        out=wq_sb[:, r2, 2 * i : 2 * i + 2, :],
                        in_=dram_ap(
                            wq_d,
                            (r2 * 512 + i * 256) * 768,
                            [[768, 128], [98304, 2], [1, 768]],
                        ),
                    )

            load_wq(0, (nc.sync, nc.scalar))
            load_wq(1, (nc.gpsimd, nc.gpsimd))

            first_cast = [True]

            def make_xtb(w, cg):
                # xtb[c-part, r2, hh(16), ww] bf16, de-strided
                xtb = xwp.tile([128, 8, 16, 32], BF16, tag="xtb", bufs=6)
                for half in range(2):
                    xt = xwp.tile([128, 8, 32, 8], BF16, tag="xt", bufs=8)
                    for hh in range(8):
                        if w == 0 and cg <= 1:
                            eng = (nc.sync, nc.scalar)[hh % 2]
                        else:
                            eng = (nc.sync, nc.scalar, nc.sync, nc.gpsimd)[hh % 4]
                        eng.dma_start(
                            out=xt[:, hh, :, :],
                            in_=dram_ap(
                                x_d,
                                cg * 16 * 65536
                                + (w * 16 + half * 8 + hh) * 2048,
                                [[65536, 16], [1, 2048]],
                            ),
                        )
                    # cast f32->bf16 + de-stride (hh, ww, r2) -> (r2, hh, ww)
                    dst = xtb[:, :, half * 8 : (half + 1) * 8, :]
                    csrc = xt[:].transpose([0, 3, 1, 2])
                    nc.vector.tensor_copy(dst, csrc)
                if first_cast[0]:
                    first_cast[0] = False
                    for r2 in range(2, 8):
                        load_wq(
                            r2,
                            (nc.scalar, nc.gpsimd)
                            if r2 % 2 == 0
                            else (nc.gpsimd, nc.scalar),
                        )
                    nc.gpsimd.dma_start(
                        out=wo_sb[:],
                        in_=dram_ap(
                            wo_d, 0, [[4096, 128], [524288, 2], [1, 4096]]
                        ),
                    )
                return xtb

            for w in range(2):
                # all 8 accumulation groups (4 qk + 4 v) live in 8 banks;
                # cg outer so compute starts after the first x tile
                qks = [
                    ps1.tile([128, 512], F32, tag=f"qk{ot}", bufs=1, name=f"qk_{w}_{ot}")
                    for ot in range(4)
                ]
                vps = [
                    ps1.tile([128, 256], F32, tag=f"v{s}", bufs=1, name=f"v_{w}_{s}")
                    for s in range(4)
                ]
                for cg in range(4):
                    xtb = make_xtb(w, cg)
                    for r2 in range(8):
                        first = cg == 0 and r2 == 0
                        last = cg == 3 and r2 == 7
                        for ot in range(4):
                            nc.tensor.matmul(
                                qks[ot][:],
                                wq_sb[:, r2, cg, ot * 128 : (ot + 1) * 128],
                                xtb[:, r2, :, :],
                                start=first,
                                stop=last,
                            )
                        for s in range(4):
                            nc.tensor.matmul(
                                vps[s][:],
                                xtb[:, r2, 4 * s : 4 * s + 4, :],
                                wq_sb[:, r2, cg, 512:768],
                                start=first,
                                stop=last,
                            )
                for ot in range(4):
                    dst = qkT[:, ot, w * 512 : (w + 1) * 512]
                    if ot % 2 == 0:
                        nc.scalar.copy(dst, qks[ot][:])
                    else:
                        nc.vector.tensor_copy(dst, qks[ot][:])
                for s in range(4):
                    nc.vector.tensor_copy(
                        v_sb[:, 4 * w + s, :, 0:64],
                        vps[s][:].rearrange("p (h d) -> p h d", h=4),
                    )

        if dbg is not None:
            nc.gpsimd.dma_start(out=dbg["qkT"][:], in_=qkT[:])
            nc.gpsimd.dma_start(out=dbg["v_sb"][:], in_=v_sb[:])

        # ======================= stage 2: attention =======================
        with (
            tc.tile_pool(name="s2", bufs=1) as s2,
            tc.tile_pool(name="s3", bufs=1) as s3,
            tc.tile_pool(name="psA", bufs=1, space="PSUM") as psA,
        ):
            for hp in range(2):  # head pair: heads 2*hp, 2*hp+1
                ed = [
                    s2.tile(
                        [128, 8, 1024], BF16, tag="edots", bufs=4, name=f"ed_{hp}_{i}"
                    )
                    for i in range(2)
                ]
                oaug = [
                    psA.tile(
                        [128, 2, 512], F32, tag="oaug", bufs=2, name=f"oaug_{hp}_{i}"
                    )
                    for i in range(2)
                ]
                for mc in range(8):
                    # dots for h2=0 (rows 0-63) / h2=1 (rows 64-127) adjacent:
                    # different PE row groups run concurrently
                    dts = [
                        psA.tile(
                            [128, 2, 512], F32, tag="dt", bufs=2,
                            name=f"dt_{hp}_{mc}_{i}",
                        )
                        for i in range(2)
                    ]
                    for h2 in range(2):
                        b = h2 * 64
                        for nh in range(2):
                            nc.tensor.matmul(
                                dts[h2][:, nh, :],
                                qkT[b : b + 64, 2 + hp, mc * 128 : (mc + 1) * 128],
                                qkT[b : b + 64, hp, nh * 512 : (nh + 1) * 512],
                                start=True,
                                stop=True,
                            )
                    for h2 in range(2):
                        h = 2 * hp + h2
                        nc.scalar.activation(
                            ed[h2][:, mc, :],
                            dts[h2][:, :, :].rearrange("p a b -> p (a b)"),
                            mybir.ActivationFunctionType.Exp,
                            scale=SCALE,
                        )
                        for nh in range(2):
                            nc.tensor.matmul(
                                oaug[h2][0:68, nh, :],
                                v_sb[:, mc, h, :],
                                ed[h2][:, mc, nh * 512 : (nh + 1) * 512],
                                start=(mc == 0),
                                stop=(mc == 7),
                            )
                # normalize: out[d, n] * (1/Z[n]) ; Z = row 64 of oaug
                for h2 in range(2):
                    h = 2 * hp + h2
                    zrow = s2.tile([65, 1024], F32, tag="zrow", bufs=2)
                    nc.vector.tensor_copy(
                        zrow[64:65, :],
                        oaug[h2][64:65, :, :].rearrange("p a b -> p (a b)"),
                    )
                    # SBUF->SBUF redistribute [1,1024] -> [64,16] in one DMA
                    z16 = s2.tile([64, 16], F32, tag="z16", bufs=2)
                    nc.sync.dma_start(out=z16[:], in_=zrow[64:65, :])
                    z16r = s2.tile([64, 16], F32, tag="z16r", bufs=2)
                    nc.vector.reciprocal(z16r[:], z16[:])
                    nc.sync.dma_start(
                        out=zrc_d[h, :].rearrange("(a b) -> a b", a=64), in_=z16r[:]
                    )
                    zbc = s2.tile([64, 1024], F32, tag="zbc", bufs=2)
                    nc.sync.dma_start(
                        out=zbc[:],
                        in_=dram_ap(zrc_d, h * 1024, [[0, 64], [1, 1024]]),
                    )
                    if dbg is not None:
                        nc.sync.dma_start(out=dbg["zbc"][:, h, :], in_=zbc[:])
                    if h2 == 0:
                        nc.vector.tensor_mul(
                            outT[0:64, hp, :],
                            oaug[h2][0:64, :, :].rearrange("p a b -> p (a b)"),
                            zbc[:],
                        )
                    else:
                        onrm = s2.tile([64, 1024], BF16, tag="onrm", bufs=2)
                        nc.vector.tensor_mul(
                            onrm[:],
                            oaug[h2][0:64, :, :].rearrange("p a b -> p (a b)"),
                            zbc[:],
                        )
                        nc.sync.dma_start(out=outT[64:128, hp, :], in_=onrm[:])

        if dbg is not None:
            nc.gpsimd.dma_start(out=dbg["outT"][:], in_=outT[:])

        # ---------------- stage 3: output projection ----------------
        with (
            tc.tile_pool(name="s3b", bufs=1) as s3,
            tc.tile_pool(name="ps3", bufs=1, space="PSUM") as psA,
        ):
            for ct in range(4):
                for nq in range(4):
                    y_big = psA.tile(
                        [128, 4, 256], F32, tag="ybig", bufs=4,
                        name=f"yb_{ct}_{nq}_a",
                    )
                    y_big2 = psA.tile(
                        [128, 4, 256], F32, tag="ybig", bufs=4,
                        name=f"yb_{ct}_{nq}_b",
                    )
                    y_t = s3.tile(
                        [128, 8, 32, 8], BF16, tag="yt", bufs=6, name=f"yt_{ct}_{nq}"
                    )
                    for rh in range(2):
                        yb = y_big if rh == 0 else y_big2
                        for r4 in range(4):
                            r2 = rh * 4 + r4
                            for ic in range(2):
                                nc.tensor.matmul(
                                    yb[:, r4, :],
                                    wo_sb[
                                        :,
                                        ic,
                                        r2 * 512
                                        + ct * 128 : r2 * 512
                                        + (ct + 1) * 128,
                                    ],
                                    outT[:, ic, nq * 256 : (nq + 1) * 256],
                                    start=(r4 % 2 == 0 and ic == 0),
                                    stop=(r4 % 2 == 1 and ic == 1),
                                )
                        dst = y_t[:, :, :, rh * 4 : (rh + 1) * 4].transpose(
                            [0, 3, 1, 2]
                        )
                        esrc = yb[:].rearrange("p r (a b) -> p r a b", a=8)
                        if zero_bias:
                            nc.vector.tensor_copy(dst, esrc)
                        else:
                            bias_bc = bias_sb[:, rh * 4 : rh * 4 + 4, ct][
                                :, :, None, None
                            ].broadcast_to([128, 4, 8, 32])
                            nc.vector.tensor_add(dst, esrc, bias_bc)
                    for hq in range(8):
                        hh = nq * 8 + hq
                        eng = (nc.sync, nc.scalar, nc.gpsimd, nc.scalar,
                               nc.sync, nc.scalar, nc.gpsimd, nc.sync)[hq]
                        eng.dma_start(
                            out=dram_ap(
                                out_d,
                                ct * 16 * 65536 + hh * 2048,
                                [[65536, 16], [1, 2048]],
                            ),
                            in_=y_t[:, hq, :, :],
                        )


def _get_nc(zero_bias=False):
    key = f"nc_zb{int(zero_bias)}"
    if key not in _CACHE:
        _CACHE[key] = _build(zero_bias=zero_bias)
    return _CACHE[key]


def _prep_weights(W_qkv, W_out, b_out):
    wq_perm = np.ascontiguousarray(
        W_qkv.reshape(64, 8, 8, 768).transpose(2, 0, 1, 3).reshape(4096, 768)
    ).astype(ml_dtypes.bfloat16)
    wo_perm = np.ascontiguousarray(
        W_out.reshape(256, 64, 8, 8).transpose(0, 3, 1, 2).reshape(256, 4096)
    ).astype(ml_dtypes.bfloat16)
    # b_perm[r2*512 + c0*8 + r1] = b_out[c0*64 + r1*8 + r2], then laid out
    # [p, r2, cg] where p = (c0 % 16)*8 + r1, cg = c0 // 16
    b_perm = b_out.reshape(64, 8, 8).transpose(2, 0, 1).reshape(4096)
    b_perm = np.ascontiguousarray(
        b_perm.reshape(8, 4, 128).transpose(2, 0, 1).reshape(4096)
    ).astype(np.float32)
    return wq_perm, wo_perm, b_perm


def kernel(x, W_qkv, W_out, b_out):
    nc = _get_nc(zero_bias=not np.any(np.asarray(b_out)))
    wq_perm, wo_perm, b_perm = _prep_weights(
        np.asarray(W_qkv, dtype=np.float32),
        np.asarray(W_out, dtype=np.float32),
        np.asarray(b_out, dtype=np.float32),
    )

    in_maps = [
        {
            "x": np.ascontiguousarray(x[b]).astype(ml_dtypes.bfloat16),
            "W_qkv": wq_perm,
            "W_out": wo_perm,
            "b_out": b_perm,
        }
        for b in range(8)
    ]
    trace = bool(int(os.environ.get("BENCH_TRACE", "0")))
    if trace:
        try:  # tracing needs the NTFF hook shim (see test.py); degrade if absent
            from antenv.axon_hooks import get_axon_ntff_profile_hook  # noqa: F401
        except ImportError:
            trace = False
    res = run_bass_kernel_spmd(nc, in_maps, core_ids=list(range(8)), trace=trace)
    if trace:
        _CACHE["last_result"] = res
    return np.stack(
        [np.asarray(res.results[b]["out"]) for b in range(8)]
    ).astype(np.float32)



# revision 9
# speedup vs baseline: 1.4423x; 1.4423x over previous
"""Trainium2 Bass kernel for PixelUnshuffle->MHA->PixelShuffle (nn_Attention).

Reference computation (per batch element, 8 batch elements data-parallel
across 8 NeuronCores):
  x [64, 256, 256] --PixelUnshuffle(8)--> tokens [N=1024, C=4096]
  qkv = tokens @ W_qkv            [1024, 768]
  4-head attention (d=64), softmax over tokens
  y = attn_out @ W_out + b_out    [1024, 4096]
  --PixelShuffle(8)--> [64, 256, 256]

Key layout trick: the pixel un/shuffle is folded into the matmuls.
x is DMA'd in its natural (fully contiguous) layout, then a cast/de-stride
engine copy produces bf16 tiles in (r2, hh, ww) order so every matmul
operand is contiguous. Weights are host-side pre-permuted + pre-cast bf16.

Token index   n = hh*32 + ww            (hh, ww in [0,32))
Channel index c = c0*64 + r1*8 + r2     (c0 in [0,64), r1, r2 in [0,8))
x[c0, hh*8+r1, ww*8+r2] = tokens[n, c]

Performance structure (v2):
 - W_qkv is loaded in 32 per-(r2,cg) chunks issued in exactly the order the
   QKV matmul loop consumes them, so the first real matmul can start ~15us
   in; W_out is deferred to the second token window. PE warmup matmuls keep
   the HAM clock-gate at 2.4 GHz until real work arrives.
 - Attention is computed transposed (dotsT[m, n], summed token m on
   partitions): dotsT = kT (lhsT) x qT -> exp -> av with a ones column in v
   accumulating the softmax denominator Z for free (row 64 of oaug).
   The mc loop is software-pipelined with av(mc-1) issued after dots(mc) so
   the in-order PE queue never stalls behind the scalar-engine exp stream
   (exp is the stage-2 bottleneck: ~(N+352)/1.2 ns, scalar is the only
   exp-capable engine). The exp activation table is preloaded at kernel
   start so no ACT_TABLE_LOAD lands on the critical path.
 - 1/Z is computed per (n-half, head-pair) via a [1,1024]->[64,16] SBUF
   redistribute DMA + reciprocal + DRAM round trip for a 0-stride partition
   broadcast (gpsimd partition_broadcast is broken for nonzero base
   partitions; 0-stride partition APs are DRAM-source only), overlapping
   the next attention block.
 - Output projection accumulates all 8 r2 blocks of a (nq, ct) tile in one
   4-bank PSUM tile, evacuated by a single tensor_copy with contiguous
   bf16 writes (strided PSUM reads), then written out as bf16 with 16KB
   contiguous DMA runs. The f32 upcast happens on the host.
"""

import sys

if "/opt/trn_rl_repo" not in sys.path:
    sys.path.insert(0, "/opt/trn_rl_repo")

import os

import ml_dtypes
import numpy as np

import concourse.bass as bass
from concourse import bacc, mybir, tile
from concourse.bass_utils import run_bass_kernel_spmd

F32 = mybir.dt.float32
BF16 = mybir.dt.bfloat16

SCALE = 0.125  # DIM_HEAD ** -0.5

_CACHE = {}


def _build(debug_outs=False, zero_bias=False):
    nc = bacc.Bacc("TRN2", target_bir_lowering=False, debug=False, num_devices=8)

    x_d = nc.dram_tensor("x", [64, 256, 256], BF16, kind="ExternalInput").ap()
    wq_d = nc.dram_tensor("W_qkv", [4096, 768], BF16, kind="ExternalInput").ap()
    wo_d = nc.dram_tensor("W_out", [256, 4096], BF16, kind="ExternalInput").ap()
    b_d = nc.dram_tensor("b_out", [4096], F32, kind="ExternalInput").ap()
    out_d = nc.dram_tensor("out", [64, 256, 256], BF16, kind="ExternalOutput").ap()

    zrc_d = nc.dram_tensor("zr_scratch", [4, 1024], F32).ap()

    dbg = None
    if debug_outs:
        dbg = {
            "qkT": nc.dram_tensor(
                "dbg_qkT", [128, 4, 1024], F32, kind="ExternalOutput"
            ).ap(),
            "v_sb": nc.dram_tensor(
                "dbg_v", [128, 8, 4, 68], F32, kind="ExternalOutput"
            ).ap(),
            "outT": nc.dram_tensor(
                "dbg_outT", [128, 2, 1024], F32, kind="ExternalOutput"
            ).ap(),
        }

    def dram_ap(base, off, pattern):
        return bass.AP(tensor=base.tensor, offset=base.offset + off, ap=pattern)

    with tile.TileContext(nc) as tc:
        _build_tiled(nc, tc, x_d, wq_d, wo_d, b_d, out_d, zrc_d, dram_ap, dbg, zero_bias)
    nc.compile()
    return nc


def _build_tiled(nc, tc, x_d, wq_d, wo_d, b_d, out_d, zrc_d, dram_ap, dbg=None, zero_bias=False):
    from contextlib import ExitStack

    with ExitStack() as ctx:
        pers = ctx.enter_context(tc.tile_pool(name="pers", bufs=1))
        s23 = ctx.enter_context(tc.tile_pool(name="s23", bufs=1))

        # ---- persistent tiles ----
        # qkT[d-part, ot, n] : ot 0,1 = q dims 0..128,128..256; ot 2,3 = k
        qkT = pers.tile([128, 4, 1024], BF16)
        # v_aug[m-part, mc, h, 68] bf16, col 64 = ones (65-67 pad for align)
        v_sb = pers.tile([128, 8, 4, 68], BF16)
        # outT[i-part, ic, n] : i = h*64+d ; ic = i//128
        outT = pers.tile([128, 2, 1024], BF16)
        # bias[c-part, r2, cg]
        bias_sb = pers.tile([128, 8, 4], F32)
        # W_out tile in outer pool; DMA issued during window-1 staging
        wo_sb = s23.tile([128, 2, 4096], BF16)  # [i-part, ic, c_perm]

        nc.vector.memset(v_sb[:, :, :, 64:68], 1.0)
        # preload the exp activation table off the critical path: one tiny
        # exp on a zeroed tile triggers ACT_TABLE_LOAD during startup
        et_in = pers.tile([64, 16], F32)
        et_out = pers.tile([64, 16], F32)
        nc.vector.memset(et_in[:], 0.0)
        nc.scalar.activation(
            et_out[:], et_in[:], mybir.ActivationFunctionType.Exp, scale=SCALE
        )
        # host pre-arranges b_out as [p, r2, cg] so this is a flat copy
        nc.gpsimd.dma_start(
            out=bias_sb[:],
            in_=dram_ap(b_d, 0, [[32, 128], [4, 8], [1, 4]]),
        )

        # =========================== stage 1 ===========================
        # QKV projection with pixel-unshuffle folded in. 2 windows of 512
        # tokens (hh-halves); x staged per half-window (8 hh rows).
        with (
            tc.tile_pool(name="wq", bufs=1) as wqp,
            tc.tile_pool(name="xw", bufs=1) as xwp,
            tc.tile_pool(name="ps1", bufs=1, space="PSUM") as ps1,
        ):
            wq_sb = wqp.tile([128, 8, 4, 768], BF16)  # [c-part, r2, cg, o]

            # PE warmup: ~10us of dummy matmuls so HAM reaches 2.4 GHz
            # before the first real matmul arrives
            warm = wqp.tile([128, 512], BF16)
            nc.vector.memset(warm[:], 0.0)
            warm_ps = ps1.tile([128, 512], F32, tag="qk0", bufs=1)
            for i in range(24):
                nc.tensor.matmul(
                    warm_ps[:], warm[:, 0:128], warm[:], start=True, stop=True
                )

            def load_wq(r2, cg, eng):
                # one (r2, cg) chunk: 128 rows x 768, issued in the order
                # the matmul loop consumes them
                eng.dma_start(
                    out=wq_sb[:, r2, cg, :],
                    in_=dram_ap(
                        wq_d,
                        (r2 * 512 + cg * 128) * 768,
                        [[768, 128], [1, 768]],
                    ),
                )

            # cg=0 chunks go first (gpsimd queue, in r2 order); x for the
            # first half-window is issued concurrently by make_xtb below
            for r2 in range(8):
                load_wq(r2, 0, nc.gpsimd)

            first_cast = [True]
            wo_loaded = [False]

            def make_xtb(w, cg):
                # xtb[c-part, r2, hh(16), ww] bf16, de-strided
                xtb = xwp.tile([128, 8, 16, 32], BF16, tag="xtb", bufs=6)
                for half in range(2):
                    xt = xwp.tile([128, 8, 32, 8], BF16, tag="xt", bufs=8)
                    for hh in range(8):
                        if w == 0 and cg <= 1:
                            eng = (nc.sync, nc.scalar)[hh % 2]
                        else:
                            eng = (nc.sync, nc.scalar, nc.sync, nc.gpsimd)[hh % 4]
                        eng.dma_start(
                            out=xt[:, hh, :, :],
                            in_=dram_ap(
                                x_d,
                                cg * 16 * 65536
                                + (w * 16 + half * 8 + hh) * 2048,
                                [[65536, 16], [1, 2048]],
                            ),
                        )
                    # de-stride (hh, ww, r2) -> (r2, hh, ww); first tile is
                    # startup-critical so its halves run scalar || vector
                    dst = xtb[:, :, half * 8 : (half + 1) * 8, :]
                    csrc = xt[:].transpose([0, 3, 1, 2])
                    if w == 0 and cg == 0 and half == 0:
                        nc.scalar.copy(dst, csrc)
                    else:
                        nc.vector.tensor_copy(dst, csrc)
                if first_cast[0]:
                    first_cast[0] = False
                    engs = (nc.sync, nc.scalar, nc.gpsimd)
                    k = 0
                    for cgl in range(1, 4):
                        for r2 in range(8):
                            load_wq(r2, cgl, engs[k % 3])
                            k += 1
                if w == 1 and not wo_loaded[0]:
                    wo_loaded[0] = True
                    for ic in range(2):
                        (nc.gpsimd, nc.scalar)[ic].dma_start(
                            out=wo_sb[:, ic, :],
                            in_=dram_ap(
                                wo_d, ic * 524288, [[4096, 128], [1, 4096]]
                            ),
                        )
                return xtb

            for w in range(2):
                # all 8 accumulation groups (4 qk + 4 v) live in 8 banks;
                # cg outer so compute starts after the first x tile
                qks = [
                    ps1.tile([128, 512], F32, tag=f"qk{ot}", bufs=1, name=f"qk_{w}_{ot}")
                    for ot in range(4)
                ]
                vps = [
                    ps1.tile([128, 256], F32, tag=f"v{s}", bufs=1, name=f"v_{w}_{s}")
                    for s in range(4)
                ]
                for cg in range(4):
                    xtb = make_xtb(w, cg)
                    for r2 in range(8):
                        first = cg == 0 and r2 == 0
                        last = cg == 3 and r2 == 7
                        for ot in range(4):
                            nc.tensor.matmul(
                                qks[ot][:],
                                wq_sb[:, r2, cg, ot * 128 : (ot + 1) * 128],
                                xtb[:, r2, :, :],
                                start=first,
                                stop=last,
                            )
                        for s in range(4):
                            nc.tensor.matmul(
                                vps[s][:],
                                xtb[:, r2, 4 * s : 4 * s + 4, :],
                                wq_sb[:, r2, cg, 512:768],
                                start=first,
                                stop=last,
                            )
                # k evacuations (ot 2,3) first: stage-2 dots for m-chunks
                # 4-7 need them soonest; q(w1) is needed later (nh=1)
                for ot in (2, 3, 0, 1):
                    dst = qkT[:, ot, w * 512 : (w + 1) * 512]
                    if ot % 2 == 0:
                        nc.scalar.copy(dst, qks[ot][:])
                    else:
                        nc.vector.tensor_copy(dst, qks[ot][:])
                for s in range(4):
                    nc.vector.tensor_copy(
                        v_sb[:, 4 * w + s, :, 0:64],
                        vps[s][:].rearrange("p (h d) -> p h d", h=4),
                    )

        if dbg is not None:
            nc.gpsimd.dma_start(out=dbg["qkT"][:], in_=qkT[:])
            nc.gpsimd.dma_start(out=dbg["v_sb"][:], in_=v_sb[:])

        # ======================= stage 2: attention =======================
        # Loops: n-half (nh) outer, head-pair (hp), summed-chunk (mc) inner.
        # PE issue order pipelines: av(mc-1) goes after dots(mc) so the PE
        # always has ready work while the scalar engine streams exps.
        with (
            tc.tile_pool(name="s2", bufs=1) as s2,
            tc.tile_pool(name="psA", bufs=1, space="PSUM") as psA,
        ):
            for nh in range(2):
                for hp in range(2):
                    oaug = [
                        psA.tile(
                            [128, 512], F32, tag=f"oa{h2}", bufs=2,
                            name=f"oaug_{nh}_{hp}_{h2}",
                        )
                        for h2 in range(2)
                    ]
                    dts_q = []
                    ed_q = []
                    for mc in range(9):
                        if mc < 8:
                            dts = psA.tile(
                                [128, 2, 512], F32, tag="dt", bufs=2,
                                name=f"dt_{nh}_{hp}_{mc}",
                            )
                            for h2 in range(2):
                                b = h2 * 64
                                nc.tensor.matmul(
                                    dts[:, h2, :],
                                    qkT[b : b + 64, 2 + hp, mc * 128 : (mc + 1) * 128],
                                    qkT[b : b + 64, hp, nh * 512 : (nh + 1) * 512],
                                    start=True,
                                    stop=True,
                                )
                            ed = s2.tile(
                                [128, 2, 512], BF16, tag="ed", bufs=3,
                                name=f"ed_{nh}_{hp}_{mc}",
                            )
                            nc.scalar.activation(
                                ed[:].rearrange("p a b -> p (a b)"),
                                dts[:].rearrange("p a b -> p (a b)"),
                                mybir.ActivationFunctionType.Exp,
                                scale=SCALE,
                            )
                            ed_q.append(ed)
                        if mc >= 1:
                            edp = ed_q[mc - 1]
                            for h2 in range(2):
                                h = 2 * hp + h2
                                nc.tensor.matmul(
                                    oaug[h2][0:68, :],
                                    v_sb[:, mc - 1, h, :],
                                    edp[:, h2, :],
                                    start=(mc == 1),
                                    stop=(mc == 8),
                                )
                    # ---- normalize this (nh, hp) block: out *= 1/Z ----
                    # Z = row 64 of oaug; batched for both heads
                    slot = nh * 2 + hp
                    zcat = s2.tile([65, 1024], F32, tag="zcat", bufs=2)
                    for h2 in range(2):
                        nc.vector.tensor_copy(
                            zcat[64:65, h2 * 512 : (h2 + 1) * 512],
                            oaug[h2][64:65, :],
                        )
                    z64 = s2.tile([64, 16], F32, tag="z64", bufs=2)
                    nc.sync.dma_start(out=z64[:], in_=zcat[64:65, :])
                    z64r = s2.tile([64, 16], F32, tag="z64r", bufs=2)
                    nc.vector.reciprocal(z64r[:], z64[:])
                    nc.sync.dma_start(
                        out=zrc_d[slot, :].rearrange("(a b) -> a b", a=64),
                        in_=z64r[:],
                    )
                    for h2 in range(2):
                        zbc = s2.tile([64, 512], F32, tag=f"zbc{h2}", bufs=2)
                        nc.sync.dma_start(
                            out=zbc[:],
                            in_=dram_ap(
                                zrc_d, slot * 1024 + h2 * 512, [[0, 64], [1, 512]]
                            ),
                        )
                        if h2 == 0:
                            nc.vector.tensor_mul(
                                outT[0:64, hp, nh * 512 : (nh + 1) * 512],
                                oaug[h2][0:64, :],
                                zbc[:],
                            )
                        else:
                            onrm = s2.tile([64, 512], BF16, tag="onrm", bufs=2)
                            nc.vector.tensor_mul(onrm[:], oaug[h2][0:64, :], zbc[:])
                            nc.sync.dma_start(
                                out=outT[64:128, hp, nh * 512 : (nh + 1) * 512],
                                in_=onrm[:],
                            )

        if dbg is not None:
            nc.gpsimd.dma_start(out=dbg["outT"][:], in_=outT[:])

        # ---------------- stage 3: output projection ----------------
        # One 4-bank PSUM tile holds all 8 r2 blocks of a (nq, ct) tile;
        # a single contiguous-bf16-write copy evacuates it; output DMAs
        # move 16KB contiguous runs (4 hh rows each).
        with (
            tc.tile_pool(name="s3b", bufs=1) as s3,
            tc.tile_pool(name="ps3", bufs=1, space="PSUM") as ps3,
        ):
            dmae = 0
            for nq in range(4):
                for ct in range(4):
                    y_ps = ps3.tile(
                        [128, 8, 256], F32, tag="yps", bufs=2,
                        name=f"yps_{nq}_{ct}",
                    )
                    for r2 in range(8):
                        for ic in range(2):
                            nc.tensor.matmul(
                                y_ps[:, r2, :],
                                wo_sb[
                                    :,
                                    ic,
                                    r2 * 512 + ct * 128 : r2 * 512 + (ct + 1) * 128,
                                ],
                                outT[:, ic, nq * 256 : (nq + 1) * 256],
                                start=(r2 % 2 == 0 and ic == 0),
                                stop=(r2 % 2 == 1 and ic == 1),
                            )
                    y_t = s3.tile(
                        [128, 8, 32, 8], BF16, tag="yt", bufs=4,
                        name=f"yt_{nq}_{ct}",
                    )
                    esrc = y_ps[:].rearrange("p r (h w) -> p h w r", h=8)
                    if zero_bias:
                        # gpsimd has no PSUM port: alternate vector/scalar
                        if ct % 2 == 0:
                            nc.vector.tensor_copy(y_t[:], esrc)
                        else:
                            nc.scalar.copy(y_t[:], esrc)
                    else:
                        bias_bc = bias_sb[:, :, ct][:, None, None, :].broadcast_to(
                            [128, 8, 32, 8]
                        )
                        nc.vector.tensor_add(y_t[:], esrc, bias_bc)
                    for hq in range(8):
                        hh = nq * 8 + hq
                        deng = (nc.sync, nc.scalar, nc.gpsimd)[dmae % 3]
                        dmae += 1
                        deng.dma_start(
                            out=dram_ap(
                                out_d,
                                ct * 16 * 65536 + hh * 2048,
                                [[65536, 16], [1, 2048]],
                            ),
                            in_=y_t[:, hq, :, :],
                        )


def _get_nc(zero_bias=False):
    key = f"nc_zb{int(zero_bias)}"
    if key not in _CACHE:
        _CACHE[key] = _build(zero_bias=zero_bias)
    return _CACHE[key]


def _prep_weights(W_qkv, W_out, b_out):
    wq_perm = np.ascontiguousarray(
        W_qkv.reshape(64, 8, 8, 768).transpose(2, 0, 1, 3).reshape(4096, 768)
    ).astype(ml_dtypes.bfloat16)
    wo_perm = np.ascontiguousarray(
        W_out.reshape(256, 64, 8, 8).transpose(0, 3, 1, 2).reshape(256, 4096)
    ).astype(ml_dtypes.bfloat16)
    # b_perm[r2*512 + c0*8 + r1] = b_out[c0*64 + r1*8 + r2], then laid out
    # [p, r2, cg] where p = (c0 % 16)*8 + r1, cg = c0 // 16
    b_perm = b_out.reshape(64, 8, 8).transpose(2, 0, 1).reshape(4096)
    b_perm = np.ascontiguousarray(
        b_perm.reshape(8, 4, 128).transpose(2, 0, 1).reshape(4096)
    ).astype(np.float32)
    return wq_perm, wo_perm, b_perm


def kernel(x, W_qkv, W_out, b_out):
    nc = _get_nc(zero_bias=not np.any(np.asarray(b_out)))
    wq_perm, wo_perm, b_perm = _prep_weights(
        np.asarray(W_qkv, dtype=np.float32),
        np.asarray(W_out, dtype=np.float32),
        np.asarray(b_out, dtype=np.float32),
    )

    in_maps = [
        {
            "x": np.ascontiguousarray(x[b]).astype(ml_dtypes.bfloat16),
            "W_qkv": wq_perm,
            "W_out": wo_perm,
            "b_out": b_perm,
        }
        for b in range(8)
    ]
    trace = bool(int(os.environ.get("BENCH_TRACE", "0")))
    if trace:
        try:  # tracing needs the NTFF hook shim (see test.py); degrade if absent
            from antenv.axon_hooks import get_axon_ntff_profile_hook  # noqa: F401
        except ImportError:
            trace = False
    res = run_bass_kernel_spmd(nc, in_maps, core_ids=list(range(8)), trace=trace)
    if trace:
        _CACHE["last_result"] = res
    return np.stack(
        [np.asarray(res.results[b]["out"]) for b in range(8)]
    ).astype(np.float32)


# revision 10
# speedup vs baseline: 1.7103x; 1.1858x over previous
"""Trainium2 Bass kernel for PixelUnshuffle->MHA->PixelShuffle (nn_Attention).

Reference computation (per batch element, 8 batch elements data-parallel
across 8 NeuronCores):
  x [64, 256, 256] --PixelUnshuffle(8)--> tokens [N=1024, C=4096]
  qkv = tokens @ W_qkv            [1024, 768]
  4-head attention (d=64), softmax over tokens
  y = attn_out @ W_out + b_out    [1024, 4096]
  --PixelShuffle(8)--> [64, 256, 256]

Layout strategy (v3): ALL data reshuffling happens on the host. x is
pre-packed (and pre-cast to bf16) into the exact [w, cg, p, r2, hh, ww]
tile layout the QKV matmul consumes, so the kernel issues just 8 fully
contiguous 1 MB input DMAs and zero de-stride copies. The output is
written as raw [nq, ct, p, hq, ww, r2] tiles (16 contiguous 512 KB DMAs)
and pixel-shuffled + upcast to f32 on the host. DMA-issue instructions
(~0.6us of engine time each) were the stage-3 bottleneck before this.

Token index   n = hh*32 + ww            (hh, ww in [0,32))
Channel index c = c0*64 + r1*8 + r2     (c0 in [0,64), r1, r2 in [0,8))
partition p = (c0 % 16)*8 + r1 within a cg/ct block of 16 c0's

Performance structure:
 - W_qkv arrives in 16 half-chunks ordered exactly as the matmul loop
   consumes them; W_out is deferred to the second token window. PE warmup
   matmuls hold the HAM clock-gate at 2.4 GHz until real work arrives
   (the PE drops to 1.2 GHz after any ~3.4us idle window).
 - Attention is computed transposed (dotsT[m, n], summed token m on
   partitions): dotsT = kT (lhsT) x qT -> exp -> av, with a ones column in
   v accumulating the softmax denominator Z for free (row 64 of oaug).
   The mc loop is software-pipelined with av(mc-1) issued after dots(mc)
   so the in-order PE queue never stalls behind the scalar-engine exp
   stream (exp is the stage-2 floor: (N+352)/1.2 ns, scalar is the only
   exp-capable engine). The exp table is preloaded at kernel start.
 - 1/Z per (n-half, head-pair): [1,1024]->[64,16] SBUF redistribute DMA,
   reciprocal, DRAM round trip for a 0-stride partition broadcast
   (partition_broadcast is broken for nonzero base partitions; 0-stride
   partition APs are DRAM-source only), overlapping the next block.
 - Output projection accumulates all 8 r2 blocks of a (nq, ct) tile in
   one 4-bank PSUM tile (dt tag allocated first so stage-3 PSUM reuses
   the dts banks, which free early). outT is split per n-half so stage-3
   nq 0/1 only waits on the first half's normalize. Evacuation is a
   single strided-read/contiguous-bf16-write copy, alternating
   vector/scalar.
"""

import sys

if "/opt/trn_rl_repo" not in sys.path:
    sys.path.insert(0, "/opt/trn_rl_repo")

import os

import ml_dtypes
import numpy as np

import concourse.bass as bass
from concourse import bacc, mybir, tile
from concourse.bass_utils import run_bass_kernel_spmd

F32 = mybir.dt.float32
BF16 = mybir.dt.bfloat16

SCALE = 0.125  # DIM_HEAD ** -0.5

_CACHE = {}


def _build(debug_outs=False, zero_bias=False):
    nc = bacc.Bacc("TRN2", target_bir_lowering=False, debug=False, num_devices=8)

    # x pre-packed on host: [w*4+cg, p, r2*2048 + hh*32 + ww]
    x_d = nc.dram_tensor("x", [8, 128, 4096], BF16, kind="ExternalInput").ap()
    wq_d = nc.dram_tensor("W_qkv", [4096, 768], BF16, kind="ExternalInput").ap()
    wo_d = nc.dram_tensor("W_out", [256, 4096], BF16, kind="ExternalInput").ap()
    b_d = nc.dram_tensor("b_out", [4096], F32, kind="ExternalInput").ap()
    # raw output tiles: [nq*4+ct, p, hq*256 + ww*8 + r2]; host pixel-shuffles
    out_d = nc.dram_tensor("out", [16, 128, 2048], BF16, kind="ExternalOutput").ap()

    zrc_d = nc.dram_tensor("zr_scratch", [4, 1024], F32).ap()

    dbg = None
    if debug_outs:
        dbg = {
            "qkT": nc.dram_tensor(
                "dbg_qkT", [128, 4, 1024], F32, kind="ExternalOutput"
            ).ap(),
            "v_sb": nc.dram_tensor(
                "dbg_v", [128, 8, 4, 68], F32, kind="ExternalOutput"
            ).ap(),
            "outT": nc.dram_tensor(
                "dbg_outT", [128, 2, 2, 512], F32, kind="ExternalOutput"
            ).ap(),
        }

    def dram_ap(base, off, pattern):
        return bass.AP(tensor=base.tensor, offset=base.offset + off, ap=pattern)

    with tile.TileContext(nc) as tc:
        _build_tiled(nc, tc, x_d, wq_d, wo_d, b_d, out_d, zrc_d, dram_ap, dbg, zero_bias)
    nc.compile()
    return nc


def _build_tiled(nc, tc, x_d, wq_d, wo_d, b_d, out_d, zrc_d, dram_ap, dbg=None, zero_bias=False):
    from contextlib import ExitStack

    with ExitStack() as ctx:
        pers = ctx.enter_context(tc.tile_pool(name="pers", bufs=1))
        s23 = ctx.enter_context(tc.tile_pool(name="s23", bufs=1))

        # ---- persistent tiles ----
        # qkT[d-part, ot, n] : ot 0,1 = q dims 0..128,128..256; ot 2,3 = k
        qkT = pers.tile([128, 4, 1024], BF16)
        # v_aug[m-part, mc, h, 68] bf16, col 64 = ones (65-67 pad for align)
        v_sb = pers.tile([128, 8, 4, 68], BF16)
        # outT[i-part, ic, n-half] split per nh for fine-grained stage-3 deps
        outT = [pers.tile([128, 2, 512], BF16, name=f"outT{nh}") for nh in range(2)]
        # bias[c-part, r2, cg]
        bias_sb = pers.tile([128, 8, 4], F32)
        # W_out tile in outer pool; DMA issued during window-1 staging
        wo_sb = s23.tile([128, 2, 4096], BF16)  # [i-part, ic, c_perm]

        nc.vector.memset(v_sb[:, :, :, 64:68], 1.0)
        # preload the exp activation table off the critical path
        et_in = pers.tile([64, 16], F32)
        et_out = pers.tile([64, 16], F32)
        nc.vector.memset(et_in[:], 0.0)
        nc.scalar.activation(
            et_out[:], et_in[:], mybir.ActivationFunctionType.Exp, scale=SCALE
        )
        # host pre-arranges b_out as [p, r2, cg] so this is a flat copy
        nc.gpsimd.dma_start(
            out=bias_sb[:],
            in_=dram_ap(b_d, 0, [[32, 128], [4, 8], [1, 4]]),
        )

        # =========================== stage 1 ===========================
        # QKV projection; 2 windows of 512 tokens (hh-halves). x arrives
        # host-packed: one contiguous 1 MB DMA per (w, cg) tile.
        with (
            tc.tile_pool(name="wq", bufs=1) as wqp,
            tc.tile_pool(name="xw", bufs=1) as xwp,
            tc.tile_pool(name="ps1", bufs=1, space="PSUM") as ps1,
        ):
            wq_sb = wqp.tile([128, 8, 4, 768], BF16)  # [c-part, r2, cg, o]

            # PE warmup: ~10us of dummy matmuls so HAM is at 2.4 GHz when
            # the first real matmul arrives
            warm = wqp.tile([128, 512], BF16)
            nc.vector.memset(warm[:], 0.0)
            warm_ps = ps1.tile([128, 512], F32, tag="qk0", bufs=1)
            for i in range(24):
                nc.tensor.matmul(
                    warm_ps[:], warm[:, 0:128], warm[:], start=True, stop=True
                )

            def load_wq(r2, i, eng):
                # half-chunk: cg pair (2i, 2i+1) of one r2 block
                eng.dma_start(
                    out=wq_sb[:, r2, 2 * i : 2 * i + 2, :],
                    in_=dram_ap(
                        wq_d,
                        (r2 * 512 + i * 256) * 768,
                        [[768, 128], [98304, 2], [1, 768]],
                    ),
                )

            def load_x(w, cg, eng):
                xtb = xwp.tile(
                    [128, 8, 16, 32], BF16, tag="xtb", bufs=4, name=f"xtb_{w}_{cg}"
                )
                eng.dma_start(
                    out=xtb[:],
                    in_=dram_ap(x_d, (w * 4 + cg) * 128 * 4096, [[4096, 128], [1, 4096]]),
                )
                return xtb

            # first x tile + cg-pair-0 weights in consumption order, then
            # the rest; all issued up front (just 24 DMA instructions)
            xtbs = {}
            xtbs[(0, 0)] = load_x(0, 0, nc.sync)
            for r2 in range(8):
                load_wq(r2, 0, (nc.scalar, nc.gpsimd)[r2 % 2])
            xtbs[(0, 1)] = load_x(0, 1, nc.sync)
            for r2 in range(8):
                load_wq(r2, 1, (nc.scalar, nc.gpsimd)[r2 % 2])
            xtbs[(0, 2)] = load_x(0, 2, nc.sync)
            xtbs[(0, 3)] = load_x(0, 3, nc.scalar)

            wo_loaded = [False]

            for w in range(2):
                # 8 accumulation groups (4 qk + 4 v) live in 8 PSUM banks
                qks = [
                    ps1.tile([128, 512], F32, tag=f"qk{ot}", bufs=1, name=f"qk_{w}_{ot}")
                    for ot in range(4)
                ]
                vps = [
                    ps1.tile([128, 256], F32, tag=f"v{s}", bufs=1, name=f"v_{w}_{s}")
                    for s in range(4)
                ]
                for cg in range(4):
                    xtb = xtbs.pop((w, cg))
                    # stage the next window's tiles as buffers free up
                    if w == 0:
                        nxt = (nc.sync, nc.scalar, nc.gpsimd, nc.sync)[cg]
                        xtbs[(1, cg)] = load_x(1, cg, nxt)
                        if not wo_loaded[0] and cg == 2:
                            wo_loaded[0] = True
                            for ic in range(2):
                                (nc.gpsimd, nc.scalar)[ic].dma_start(
                                    out=wo_sb[:, ic, :],
                                    in_=dram_ap(
                                        wo_d, ic * 524288, [[4096, 128], [1, 4096]]
                                    ),
                                )
                    for r2 in range(8):
                        first = cg == 0 and r2 == 0
                        last = cg == 3 and r2 == 7
                        for ot in range(4):
                            nc.tensor.matmul(
                                qks[ot][:],
                                wq_sb[:, r2, cg, ot * 128 : (ot + 1) * 128],
                                xtb[:, r2, :, :],
                                start=first,
                                stop=last,
                            )
                        for s in range(4):
                            nc.tensor.matmul(
                                vps[s][:],
                                xtb[:, r2, 4 * s : 4 * s + 4, :],
                                wq_sb[:, r2, cg, 512:768],
                                start=first,
                                stop=last,
                            )
                # k evacuations (ot 2,3) first: stage-2 dots for m-chunks
                # 4-7 need them soonest; q(w1) is needed later (nh=1)
                for ot in (2, 3, 0, 1):
                    dst = qkT[:, ot, w * 512 : (w + 1) * 512]
                    if ot % 2 == 0:
                        nc.scalar.copy(dst, qks[ot][:])
                    else:
                        nc.vector.tensor_copy(dst, qks[ot][:])
                for s in range(4):
                    nc.vector.tensor_copy(
                        v_sb[:, 4 * w + s, :, 0:64],
                        vps[s][:].rearrange("p (h d) -> p h d", h=4),
                    )

        if dbg is not None:
            nc.gpsimd.dma_start(out=dbg["qkT"][:], in_=qkT[:])
            nc.gpsimd.dma_start(out=dbg["v_sb"][:], in_=v_sb[:])

        # ======================= stage 2: attention =======================
        # Loops: n-half (nh) outer, head-pair (hp), summed-chunk (mc) inner.
        # PE issue order pipelines: av(mc-1) goes after dots(mc) so the PE
        # always has ready work while the scalar engine streams exps.
        with (
            tc.tile_pool(name="s2", bufs=1) as s2,
            tc.tile_pool(name="psA", bufs=1, space="PSUM") as psA,
        ):
            # allocate the dt tag FIRST so it lands in the low PSUM banks;
            # stage-3's first y tile then reuses dts banks (freed early)
            # rather than oaug banks (freed only after the last normalize)
            dts0 = psA.tile([128, 2, 512], F32, tag="dt", bufs=2, name="dt_first")
            for nh in range(2):
                for hp in range(2):
                    oaug = [
                        psA.tile(
                            [128, 512], F32, tag=f"oa{h2}", bufs=2,
                            name=f"oaug_{nh}_{hp}_{h2}",
                        )
                        for h2 in range(2)
                    ]
                    ed_q = []
                    for mc in range(9):
                        if mc < 8:
                            if dts0 is not None:
                                dts, dts0 = dts0, None
                            else:
                                dts = psA.tile(
                                    [128, 2, 512], F32, tag="dt", bufs=2,
                                    name=f"dt_{nh}_{hp}_{mc}",
                                )
                            for h2 in range(2):
                                b = h2 * 64
                                nc.tensor.matmul(
                                    dts[:, h2, :],
                                    qkT[b : b + 64, 2 + hp, mc * 128 : (mc + 1) * 128],
                                    qkT[b : b + 64, hp, nh * 512 : (nh + 1) * 512],
                                    start=True,
                                    stop=True,
                                )
                            ed = s2.tile(
                                [128, 2, 512], BF16, tag="ed", bufs=3,
                                name=f"ed_{nh}_{hp}_{mc}",
                            )
                            nc.scalar.activation(
                                ed[:].rearrange("p a b -> p (a b)"),
                                dts[:].rearrange("p a b -> p (a b)"),
                                mybir.ActivationFunctionType.Exp,
                                scale=SCALE,
                            )
                            ed_q.append(ed)
                        if mc >= 1:
                            edp = ed_q[mc - 1]
                            for h2 in range(2):
                                h = 2 * hp + h2
                                nc.tensor.matmul(
                                    oaug[h2][0:68, :],
                                    v_sb[:, mc - 1, h, :],
                                    edp[:, h2, :],
                                    start=(mc == 1),
                                    stop=(mc == 8),
                                )
                    # ---- normalize this (nh, hp) block: out *= 1/Z ----
                    # Z = row 64 of oaug; batched for both heads
                    slot = nh * 2 + hp
                    zcat = s2.tile([65, 1024], F32, tag="zcat", bufs=2)
                    for h2 in range(2):
                        nc.vector.tensor_copy(
                            zcat[64:65, h2 * 512 : (h2 + 1) * 512],
                            oaug[h2][64:65, :],
                        )
                    z64 = s2.tile([64, 16], F32, tag="z64", bufs=2)
                    nc.sync.dma_start(out=z64[:], in_=zcat[64:65, :])
                    z64r = s2.tile([64, 16], F32, tag="z64r", bufs=2)
                    nc.vector.reciprocal(z64r[:], z64[:])
                    nc.sync.dma_start(
                        out=zrc_d[slot, :].rearrange("(a b) -> a b", a=64),
                        in_=z64r[:],
                    )
                    for h2 in range(2):
                        zbc = s2.tile([64, 512], F32, tag=f"zbc{h2}", bufs=2)
                        nc.sync.dma_start(
                            out=zbc[:],
                            in_=dram_ap(
                                zrc_d, slot * 1024 + h2 * 512, [[0, 64], [1, 512]]
                            ),
                        )
                        if h2 == 0:
                            nc.vector.tensor_mul(
                                outT[nh][0:64, hp, :],
                                oaug[h2][0:64, :],
                                zbc[:],
                            )
                        else:
                            onrm = s2.tile([64, 512], BF16, tag="onrm", bufs=2)
                            nc.vector.tensor_mul(onrm[:], oaug[h2][0:64, :], zbc[:])
                            nc.sync.dma_start(
                                out=outT[nh][64:128, hp, :],
                                in_=onrm[:],
                            )

        if dbg is not None:
            for nh in range(2):
                nc.gpsimd.dma_start(out=dbg["outT"][:, :, nh, :], in_=outT[nh][:])

        # ---------------- stage 3: output projection ----------------
        # One 4-bank PSUM tile holds all 8 r2 blocks of a (nq, ct) tile;
        # a single contiguous-bf16-write copy evacuates it; the output DMA
        # is one contiguous 512 KB transfer per tile (host pixel-shuffles).
        with (
            tc.tile_pool(name="s3b", bufs=1) as s3,
            tc.tile_pool(name="ps3", bufs=1, space="PSUM") as ps3,
        ):
            for nq in range(4):
                for ct in range(4):
                    y_ps = ps3.tile(
                        [128, 8, 256], F32, tag="yps", bufs=2,
                        name=f"yps_{nq}_{ct}",
                    )
                    for r2 in range(8):
                        for ic in range(2):
                            nc.tensor.matmul(
                                y_ps[:, r2, :],
                                wo_sb[
                                    :,
                                    ic,
                                    r2 * 512 + ct * 128 : r2 * 512 + (ct + 1) * 128,
                                ],
                                outT[nq // 2][
                                    :, ic, (nq % 2) * 256 : (nq % 2 + 1) * 256
                                ],
                                start=(r2 % 2 == 0 and ic == 0),
                                stop=(r2 % 2 == 1 and ic == 1),
                            )
                    y_t = s3.tile(
                        [128, 8, 32, 8], BF16, tag="yt", bufs=4,
                        name=f"yt_{nq}_{ct}",
                    )
                    esrc = y_ps[:].rearrange("p r (h w) -> p h w r", h=8)
                    if zero_bias:
                        # gpsimd has no PSUM port: alternate vector/scalar
                        if ct % 2 == 0:
                            nc.vector.tensor_copy(y_t[:], esrc)
                        else:
                            nc.scalar.copy(y_t[:], esrc)
                    else:
                        bias_bc = bias_sb[:, :, ct][:, None, None, :].broadcast_to(
                            [128, 8, 32, 8]
                        )
                        nc.vector.tensor_add(y_t[:], esrc, bias_bc)
                    deng = (nc.sync, nc.gpsimd)[(nq * 4 + ct) % 2]
                    deng.dma_start(
                        out=dram_ap(
                            out_d, (nq * 4 + ct) * 128 * 2048, [[2048, 128], [1, 2048]]
                        ),
                        in_=y_t[:],
                    )


def _get_nc(zero_bias=False):
    key = f"nc_zb{int(zero_bias)}"
    if key not in _CACHE:
        _CACHE[key] = _build(zero_bias=zero_bias)
    return _CACHE[key]


def _prep_weights(W_qkv, W_out, b_out):
    wq_perm = np.ascontiguousarray(
        W_qkv.reshape(64, 8, 8, 768).transpose(2, 0, 1, 3).reshape(4096, 768)
    ).astype(ml_dtypes.bfloat16)
    wo_perm = np.ascontiguousarray(
        W_out.reshape(256, 64, 8, 8).transpose(0, 3, 1, 2).reshape(256, 4096)
    ).astype(ml_dtypes.bfloat16)
    # b_perm[r2*512 + c0*8 + r1] = b_out[c0*64 + r1*8 + r2], then laid out
    # [p, r2, cg] where p = (c0 % 16)*8 + r1, cg = c0 // 16
    b_perm = b_out.reshape(64, 8, 8).transpose(2, 0, 1).reshape(4096)
    b_perm = np.ascontiguousarray(
        b_perm.reshape(8, 4, 128).transpose(2, 0, 1).reshape(4096)
    ).astype(np.float32)
    return wq_perm, wo_perm, b_perm


def _pack_x(xb):
    # xb [64, 256, 256] f32 -> [w*4+cg, p=(c0%16)*8+r1, r2*2048+hh*32+ww] bf16
    # x[c0, (w*16+hh)*8 + r1, ww*8 + r2]
    t = xb.astype(ml_dtypes.bfloat16)
    t = t.reshape(4, 16, 2, 16, 8, 32, 8)  # [cg, c0l, w, hh, r1, ww, r2]
    t = t.transpose(2, 0, 1, 4, 6, 3, 5)   # [w, cg, c0l, r1, r2, hh, ww]
    return np.ascontiguousarray(t.reshape(8, 128, 4096))


def _unpack_out(raw):
    # raw [16, 128, 2048] = [nq*4+ct, (c0%16)*8+r1, hq*256+ww*8+r2]
    # -> y[c0, (nq*8+hq)*8 + r1, ww*8 + r2]
    t = np.asarray(raw).reshape(4, 4, 16, 8, 8, 32, 8)  # [nq, ct, c0l, r1, hq, ww, r2]
    t = t.transpose(1, 2, 0, 4, 3, 5, 6)  # [ct, c0l, nq, hq, r1, ww, r2]
    return t.reshape(64, 256, 256)


def kernel(x, W_qkv, W_out, b_out):
    nc = _get_nc(zero_bias=not np.any(np.asarray(b_out)))
    wq_perm, wo_perm, b_perm = _prep_weights(
        np.asarray(W_qkv, dtype=np.float32),
        np.asarray(W_out, dtype=np.float32),
        np.asarray(b_out, dtype=np.float32),
    )

    in_maps = [
        {
            "x": _pack_x(np.asarray(x[b], dtype=np.float32)),
            "W_qkv": wq_perm,
            "W_out": wo_perm,
            "b_out": b_perm,
        }
        for b in range(8)
    ]
    trace = bool(int(os.environ.get("BENCH_TRACE", "0")))
    if trace:
        try:  # tracing needs the NTFF hook shim (see test.py); degrade if absent
            from antenv.axon_hooks import get_axon_ntff_profile_hook  # noqa: F401
        except ImportError:
            trace = False
    res = run_bass_kernel_spmd(nc, in_maps, core_ids=list(range(8)), trace=trace)
    if trace:
        _CACHE["last_result"] = res
    return np.stack(
        [_unpack_out(res.results[b]["out"]) for b in range(8)]
    ).astype(np.float32)


# revision 13
# speedup vs baseline: 1.7869x; 1.0448x over previous
"""Trainium2 Bass kernel for PixelUnshuffle->MHA->PixelShuffle (nn_Attention).

Reference computation (per batch element, 8 batch elements data-parallel
across 8 NeuronCores):
  x [64, 256, 256] --PixelUnshuffle(8)--> tokens [N=1024, C=4096]
  qkv = tokens @ W_qkv            [1024, 768]
  4-head attention (d=64), softmax over tokens
  y = attn_out @ W_out + b_out    [1024, 4096]
  --PixelShuffle(8)--> [64, 256, 256]

Layout strategy (v3): ALL data reshuffling happens on the host. x is
pre-packed (and pre-cast to bf16) into the exact [w, cg, p, r2, hh, ww]
tile layout the QKV matmul consumes, so the kernel issues just 8 fully
contiguous 1 MB input DMAs and zero de-stride copies. The output is
written as raw [nq, ct, p, hq, ww, r2] tiles (16 contiguous 512 KB DMAs)
and pixel-shuffled + upcast to f32 on the host. DMA-issue instructions
(~0.6us of engine time each) were the stage-3 bottleneck before this.

Token index   n = hh*32 + ww            (hh, ww in [0,32))
Channel index c = c0*64 + r1*8 + r2     (c0 in [0,64), r1, r2 in [0,8))
partition p = (c0 % 16)*8 + r1 within a cg/ct block of 16 c0's

Performance structure:
 - W_qkv arrives in 16 half-chunks ordered exactly as the matmul loop
   consumes them; W_out is deferred to the second token window. PE warmup
   matmuls hold the HAM clock-gate at 2.4 GHz until real work arrives
   (the PE drops to 1.2 GHz after any ~3.4us idle window).
 - Attention is computed transposed (dotsT[m, n], summed token m on
   partitions): dotsT = kT (lhsT) x qT -> exp -> av, with a ones column in
   v accumulating the softmax denominator Z for free (row 64 of oaug).
   The mc loop is software-pipelined with av(mc-1) issued after dots(mc)
   so the in-order PE queue never stalls behind the scalar-engine exp
   stream (exp is the stage-2 floor: (N+352)/1.2 ns, scalar is the only
   exp-capable engine). The exp table is preloaded at kernel start.
 - 1/Z per (n-half, head-pair): [1,1024]->[64,16] SBUF redistribute DMA,
   reciprocal, DRAM round trip for a 0-stride partition broadcast
   (partition_broadcast is broken for nonzero base partitions; 0-stride
   partition APs are DRAM-source only), overlapping the next block.
 - Output projection accumulates all 8 r2 blocks of a (nq, ct) tile in
   one 4-bank PSUM tile (dt tag allocated first so stage-3 PSUM reuses
   the dts banks, which free early). outT is split per n-half so stage-3
   nq 0/1 only waits on the first half's normalize. Evacuation is a
   single strided-read/contiguous-bf16-write copy, alternating
   vector/scalar.
"""

import sys

if "/opt/trn_rl_repo" not in sys.path:
    sys.path.insert(0, "/opt/trn_rl_repo")

import os

import ml_dtypes
import numpy as np

import concourse.bass as bass
from concourse import bacc, mybir, tile
from concourse.bass_utils import run_bass_kernel_spmd

F32 = mybir.dt.float32
BF16 = mybir.dt.bfloat16

SCALE = 0.125  # DIM_HEAD ** -0.5

_CACHE = {}


def _build(debug_outs=False, zero_bias=False):
    nc = bacc.Bacc("TRN2", target_bir_lowering=False, debug=False, num_devices=8)

    # x pre-packed on host: [w*4+cg, p, r2*2048 + hh*32 + ww]
    x_d = nc.dram_tensor("x", [8, 128, 4096], BF16, kind="ExternalInput").ap()
    wq_d = nc.dram_tensor("W_qkv", [4096, 768], BF16, kind="ExternalInput").ap()
    wo_d = nc.dram_tensor("W_out", [256, 4096], BF16, kind="ExternalInput").ap()
    b_d = nc.dram_tensor("b_out", [4096], F32, kind="ExternalInput").ap()
    # raw output tiles: [nq*4+ct, p, hq*256 + ww*8 + r2]; host pixel-shuffles
    out_d = nc.dram_tensor("out", [16, 128, 2048], BF16, kind="ExternalOutput").ap()

    zrc_d = nc.dram_tensor("zr_scratch", [4, 1024], F32).ap()

    dbg = None
    if debug_outs:
        dbg = {
            "qkT": nc.dram_tensor(
                "dbg_qkT", [128, 4, 1024], F32, kind="ExternalOutput"
            ).ap(),
            "v_sb": nc.dram_tensor(
                "dbg_v", [128, 8, 4, 68], F32, kind="ExternalOutput"
            ).ap(),
            "outT": nc.dram_tensor(
                "dbg_outT", [128, 2, 2, 512], F32, kind="ExternalOutput"
            ).ap(),
        }

    def dram_ap(base, off, pattern):
        return bass.AP(tensor=base.tensor, offset=base.offset + off, ap=pattern)

    with tile.TileContext(nc) as tc:
        _build_tiled(nc, tc, x_d, wq_d, wo_d, b_d, out_d, zrc_d, dram_ap, dbg, zero_bias)
    nc.compile()
    return nc


def _build_tiled(nc, tc, x_d, wq_d, wo_d, b_d, out_d, zrc_d, dram_ap, dbg=None, zero_bias=False):
    from contextlib import ExitStack

    with ExitStack() as ctx:
        pers = ctx.enter_context(tc.tile_pool(name="pers", bufs=1))
        s23 = ctx.enter_context(tc.tile_pool(name="s23", bufs=1))

        # ---- persistent tiles ----
        # qkT[d-part, ot, n] : ot 0,1 = q dims 0..128,128..256; ot 2,3 = k
        qkT = pers.tile([128, 4, 1024], BF16)
        # v_aug[m-part, mc, h, 68] bf16, col 64 = ones (65-67 pad for align)
        v_sb = pers.tile([128, 8, 4, 68], BF16)
        # outT[i-part, ic, n-half] split per nh for fine-grained stage-3 deps
        outT = [pers.tile([128, 2, 512], BF16, name=f"outT{nh}") for nh in range(2)]
        # bias[c-part, r2, cg]
        bias_sb = pers.tile([128, 8, 4], F32)
        # W_out tile in outer pool; DMA issued during window-1 staging
        wo_sb = s23.tile([128, 2, 4096], BF16)  # [i-part, ic, c_perm]

        nc.vector.memset(v_sb[:, :, :, 64:68], 1.0)
        # preload the exp activation table off the critical path
        et_in = pers.tile([64, 16], F32)
        et_out = pers.tile([64, 16], F32)
        nc.vector.memset(et_in[:], 0.0)
        nc.scalar.activation(
            et_out[:], et_in[:], mybir.ActivationFunctionType.Exp, scale=SCALE
        )
        # host pre-arranges b_out as [p, r2, cg] so this is a flat copy
        nc.gpsimd.dma_start(
            out=bias_sb[:],
            in_=dram_ap(b_d, 0, [[32, 128], [4, 8], [1, 4]]),
        )

        # =========================== stage 1 ===========================
        # QKV projection; 2 windows of 512 tokens (hh-halves). x arrives
        # host-packed: one contiguous 1 MB DMA per (w, cg) tile.
        with (
            tc.tile_pool(name="wq", bufs=1) as wqp,
            tc.tile_pool(name="xw", bufs=1) as xwp,
            tc.tile_pool(name="ps1", bufs=1, space="PSUM") as ps1,
        ):
            wq_sb = wqp.tile([128, 8, 4, 768], BF16)  # [c-part, r2, cg, o]

            # PE warmup: ~10us of dummy matmuls so HAM is at 2.4 GHz when
            # the first real matmul arrives
            warm = wqp.tile([128, 512], BF16)
            nc.vector.memset(warm[:], 0.0)
            warm_ps = ps1.tile([128, 512], F32, tag="qk0", bufs=1)
            for i in range(36):
                nc.tensor.matmul(
                    warm_ps[:], warm[:, 0:128], warm[:], start=True, stop=True
                )

            def load_wq(r2, i, eng):
                # half-chunk: cg pair (2i, 2i+1) of one r2 block
                eng.dma_start(
                    out=wq_sb[:, r2, 2 * i : 2 * i + 2, :],
                    in_=dram_ap(
                        wq_d,
                        (r2 * 512 + i * 256) * 768,
                        [[768, 128], [98304, 2], [1, 768]],
                    ),
                )

            def load_x(w, cg, eng):
                xtb = xwp.tile(
                    [128, 8, 16, 32], BF16, tag="xtb", bufs=4, name=f"xtb_{w}_{cg}"
                )
                eng.dma_start(
                    out=xtb[:],
                    in_=dram_ap(x_d, (w * 4 + cg) * 128 * 4096, [[4096, 128], [1, 4096]]),
                )
                return xtb

            # first x tile + cg-pair-0 weights in consumption order, then
            # the rest; all issued up front (just 24 DMA instructions)
            xtbs = {}
            xtbs[(0, 0)] = load_x(0, 0, nc.sync)
            for r2 in range(8):
                load_wq(r2, 0, (nc.scalar, nc.gpsimd)[r2 % 2])
            xtbs[(0, 1)] = load_x(0, 1, nc.sync)
            for r2 in range(8):
                load_wq(r2, 1, (nc.scalar, nc.gpsimd)[r2 % 2])
            xtbs[(0, 2)] = load_x(0, 2, nc.sync)
            xtbs[(0, 3)] = load_x(0, 3, nc.scalar)

            wo_loaded = [False]

            for w in range(2):
                # 8 accumulation groups (4 qk + 4 v) live in 8 PSUM banks
                qks = [
                    ps1.tile([128, 512], F32, tag=f"qk{ot}", bufs=1, name=f"qk_{w}_{ot}")
                    for ot in range(4)
                ]
                vps = [
                    ps1.tile([128, 256], F32, tag=f"v{s}", bufs=1, name=f"v_{w}_{s}")
                    for s in range(4)
                ]
                for cg in range(4):
                    xtb = xtbs.pop((w, cg))
                    # stage the next window's tiles as buffers free up
                    if w == 0:
                        nxt = (nc.sync, nc.scalar, nc.gpsimd, nc.sync)[cg]
                        xtbs[(1, cg)] = load_x(1, cg, nxt)
                        if not wo_loaded[0] and cg == 2:
                            wo_loaded[0] = True
                            for ic in range(2):
                                (nc.gpsimd, nc.scalar)[ic].dma_start(
                                    out=wo_sb[:, ic, :],
                                    in_=dram_ap(
                                        wo_d, ic * 524288, [[4096, 128], [1, 4096]]
                                    ),
                                )
                    for r2 in range(8):
                        first = cg == 0 and r2 == 0
                        last = cg == 3 and r2 == 7
                        for ot in range(4):
                            nc.tensor.matmul(
                                qks[ot][:],
                                wq_sb[:, r2, cg, ot * 128 : (ot + 1) * 128],
                                xtb[:, r2, :, :],
                                start=first,
                                stop=last,
                            )
                        for s in range(4):
                            nc.tensor.matmul(
                                vps[s][:],
                                xtb[:, r2, 4 * s : 4 * s + 4, :],
                                wq_sb[:, r2, cg, 512:768],
                                start=first,
                                stop=last,
                            )
                # k evacuations (ot 2,3) first: stage-2 dots for m-chunks
                # 4-7 need them soonest; q(w1) is needed later (nh=1)
                for ot in (2, 3, 0, 1):
                    dst = qkT[:, ot, w * 512 : (w + 1) * 512]
                    if ot % 2 == 0:
                        nc.scalar.copy(dst, qks[ot][:])
                    else:
                        nc.vector.tensor_copy(dst, qks[ot][:])
                for s in range(4):
                    nc.vector.tensor_copy(
                        v_sb[:, 4 * w + s, :, 0:64],
                        vps[s][:].rearrange("p (h d) -> p h d", h=4),
                    )

        if dbg is not None:
            nc.gpsimd.dma_start(out=dbg["qkT"][:], in_=qkT[:])
            nc.gpsimd.dma_start(out=dbg["v_sb"][:], in_=v_sb[:])

        # ======================= stage 2: attention =======================
        # Loops: n-half (nh) outer, head-pair (hp), summed-chunk (mc) inner.
        # PE issue order pipelines: av(mc-1) goes after dots(mc) so the PE
        # always has ready work while the scalar engine streams exps.
        with (
            tc.tile_pool(name="s2", bufs=1) as s2,
            tc.tile_pool(name="psA", bufs=1, space="PSUM") as psA,
        ):
            # allocate the dt tag FIRST so it lands in the low PSUM banks;
            # stage-3's first y tile then reuses dts banks (freed early)
            # rather than oaug banks (freed only after the last normalize)
            dts0 = psA.tile([128, 2, 512], F32, tag="dt", bufs=2, name="dt_first")
            for nh in range(2):
                for hp in range(2):
                    oaug = [
                        psA.tile(
                            [128, 512], F32, tag=f"oa{h2}", bufs=2,
                            name=f"oaug_{nh}_{hp}_{h2}",
                        )
                        for h2 in range(2)
                    ]
                    ed_q = []
                    for mc in range(9):
                        if mc < 8:
                            if dts0 is not None:
                                dts, dts0 = dts0, None
                            else:
                                dts = psA.tile(
                                    [128, 2, 512], F32, tag="dt", bufs=2,
                                    name=f"dt_{nh}_{hp}_{mc}",
                                )
                            for h2 in range(2):
                                b = h2 * 64
                                nc.tensor.matmul(
                                    dts[:, h2, :],
                                    qkT[b : b + 64, 2 + hp, mc * 128 : (mc + 1) * 128],
                                    qkT[b : b + 64, hp, nh * 512 : (nh + 1) * 512],
                                    start=True,
                                    stop=True,
                                )
                            ed = s2.tile(
                                [128, 2, 512], BF16, tag="ed", bufs=3,
                                name=f"ed_{nh}_{hp}_{mc}",
                            )
                            nc.scalar.activation(
                                ed[:].rearrange("p a b -> p (a b)"),
                                dts[:].rearrange("p a b -> p (a b)"),
                                mybir.ActivationFunctionType.Exp,
                                scale=SCALE,
                            )
                            ed_q.append(ed)
                        if mc >= 1:
                            edp = ed_q[mc - 1]
                            for h2 in range(2):
                                h = 2 * hp + h2
                                nc.tensor.matmul(
                                    oaug[h2][0:68, :],
                                    v_sb[:, mc - 1, h, :],
                                    edp[:, h2, :],
                                    start=(mc == 1),
                                    stop=(mc == 8),
                                )
                    # ---- normalize this (nh, hp) block: out *= 1/Z ----
                    # oaug is first evacuated to SBUF so the PSUM banks (and
                    # the psA pool at scope close) free right after the last
                    # av matmul rather than after the Z DMA round trip.
                    # Normalize tiles live in the outer s23 pool for the
                    # same reason. Z = row 64 of oev.
                    slot = nh * 2 + hp
                    oev = s23.tile([65, 2, 512], F32, tag="oev", bufs=2)
                    for h2 in range(2):
                        nc.vector.tensor_copy(oev[:, h2, :], oaug[h2][0:65, :])
                    z64 = s23.tile([64, 16], F32, tag="z64", bufs=2)
                    nc.sync.dma_start(out=z64[:], in_=oev[64:65, :, :])
                    z64r = s23.tile([64, 16], F32, tag="z64r", bufs=2)
                    nc.vector.reciprocal(z64r[:], z64[:])
                    nc.sync.dma_start(
                        out=zrc_d[slot, :].rearrange("(a b) -> a b", a=64),
                        in_=z64r[:],
                    )
                    for h2 in range(2):
                        zbc = s23.tile([64, 512], F32, tag=f"zbc{h2}", bufs=2)
                        nc.sync.dma_start(
                            out=zbc[:],
                            in_=dram_ap(
                                zrc_d, slot * 1024 + h2 * 512, [[0, 64], [1, 512]]
                            ),
                        )
                        if h2 == 0:
                            nc.vector.tensor_mul(
                                outT[nh][0:64, hp, :],
                                oev[0:64, h2, :],
                                zbc[:],
                            )
                        else:
                            onrm = s23.tile([64, 512], BF16, tag="onrm", bufs=2)
                            nc.vector.tensor_mul(onrm[:], oev[0:64, h2, :], zbc[:])
                            nc.sync.dma_start(
                                out=outT[nh][64:128, hp, :],
                                in_=onrm[:],
                            )

        if dbg is not None:
            for nh in range(2):
                nc.gpsimd.dma_start(out=dbg["outT"][:, :, nh, :], in_=outT[nh][:])

        # ---------------- stage 3: output projection ----------------
        # One 4-bank PSUM tile holds all 8 r2 blocks of a (nq, ct) tile;
        # a single contiguous-bf16-write copy evacuates it; the output DMA
        # is one contiguous 512 KB transfer per tile (host pixel-shuffles).
        with (
            tc.tile_pool(name="s3b", bufs=1) as s3,
            tc.tile_pool(name="ps3", bufs=1, space="PSUM") as ps3,
        ):
            for nq in range(4):
                for ct in range(4):
                    y_ps = ps3.tile(
                        [128, 8, 256], F32, tag="yps", bufs=2,
                        name=f"yps_{nq}_{ct}",
                    )
                    for r2 in range(8):
                        for ic in range(2):
                            nc.tensor.matmul(
                                y_ps[:, r2, :],
                                wo_sb[
                                    :,
                                    ic,
                                    r2 * 512 + ct * 128 : r2 * 512 + (ct + 1) * 128,
                                ],
                                outT[nq // 2][
                                    :, ic, (nq % 2) * 256 : (nq % 2 + 1) * 256
                                ],
                                start=(r2 % 2 == 0 and ic == 0),
                                stop=(r2 % 2 == 1 and ic == 1),
                            )
                    y_t = s3.tile(
                        [128, 8, 32, 8], BF16, tag="yt", bufs=4,
                        name=f"yt_{nq}_{ct}",
                    )
                    # evacuate in halves on vector || scalar (gpsimd has no
                    # PSUM port) and DMA each half as soon as it's ready
                    for half in range(2):
                        dst = y_t[:, half * 4 : (half + 1) * 4, :, :]
                        esrc = y_ps[:, :, half * 128 : (half + 1) * 128].rearrange(
                            "p r (h w) -> p h w r", h=4
                        )
                        if zero_bias:
                            if half == 0:
                                nc.vector.tensor_copy(dst, esrc)
                            else:
                                nc.scalar.copy(dst, esrc)
                        else:
                            bias_bc = bias_sb[:, :, ct][
                                :, None, None, :
                            ].broadcast_to([128, 4, 32, 8])
                            nc.vector.tensor_add(dst, esrc, bias_bc)
                        deng = (nc.sync, nc.gpsimd)[half]
                        deng.dma_start(
                            out=dram_ap(
                                out_d,
                                (nq * 4 + ct) * 128 * 2048 + half * 1024,
                                [[2048, 128], [1, 1024]],
                            ),
                            in_=y_t[:, half * 4 : (half + 1) * 4, :, :],
                        )


def _get_nc(zero_bias=False):
    key = f"nc_zb{int(zero_bias)}"
    if key not in _CACHE:
        _CACHE[key] = _build(zero_bias=zero_bias)
    return _CACHE[key]


def _prep_weights(W_qkv, W_out, b_out):
    wq_perm = np.ascontiguousarray(
        W_qkv.reshape(64, 8, 8, 768).transpose(2, 0, 1, 3).reshape(4096, 768)
    ).astype(ml_dtypes.bfloat16)
    wo_perm = np.ascontiguousarray(
        W_out.reshape(256, 64, 8, 8).transpose(0, 3, 1, 2).reshape(256, 4096)
    ).astype(ml_dtypes.bfloat16)
    # b_perm[r2*512 + c0*8 + r1] = b_out[c0*64 + r1*8 + r2], then laid out
    # [p, r2, cg] where p = (c0 % 16)*8 + r1, cg = c0 // 16
    b_perm = b_out.reshape(64, 8, 8).transpose(2, 0, 1).reshape(4096)
    b_perm = np.ascontiguousarray(
        b_perm.reshape(8, 4, 128).transpose(2, 0, 1).reshape(4096)
    ).astype(np.float32)
    return wq_perm, wo_perm, b_perm


def _pack_x(xb):
    # xb [64, 256, 256] f32 -> [w*4+cg, p=(c0%16)*8+r1, r2*2048+hh*32+ww] bf16
    # x[c0, (w*16+hh)*8 + r1, ww*8 + r2]
    t = xb.astype(ml_dtypes.bfloat16)
    t = t.reshape(4, 16, 2, 16, 8, 32, 8)  # [cg, c0l, w, hh, r1, ww, r2]
    t = t.transpose(2, 0, 1, 4, 6, 3, 5)   # [w, cg, c0l, r1, r2, hh, ww]
    return np.ascontiguousarray(t.reshape(8, 128, 4096))


def _unpack_out(raw):
    # raw [16, 128, 2048] = [nq*4+ct, (c0%16)*8+r1, hq*256+ww*8+r2]
    # -> y[c0, (nq*8+hq)*8 + r1, ww*8 + r2]
    t = np.asarray(raw).reshape(4, 4, 16, 8, 8, 32, 8)  # [nq, ct, c0l, r1, hq, ww, r2]
    t = t.transpose(1, 2, 0, 4, 3, 5, 6)  # [ct, c0l, nq, hq, r1, ww, r2]
    return t.reshape(64, 256, 256)


def kernel(x, W_qkv, W_out, b_out):
    nc = _get_nc(zero_bias=not np.any(np.asarray(b_out)))
    wq_perm, wo_perm, b_perm = _prep_weights(
        np.asarray(W_qkv, dtype=np.float32),
        np.asarray(W_out, dtype=np.float32),
        np.asarray(b_out, dtype=np.float32),
    )

    in_maps = [
        {
            "x": _pack_x(np.asarray(x[b], dtype=np.float32)),
            "W_qkv": wq_perm,
            "W_out": wo_perm,
            "b_out": b_perm,
        }
        for b in range(8)
    ]
    trace = bool(int(os.environ.get("BENCH_TRACE", "0")))
    if trace:
        try:  # tracing needs the NTFF hook shim (see test.py); degrade if absent
            from antenv.axon_hooks import get_axon_ntff_profile_hook  # noqa: F401
        except ImportError:
            trace = False
    res = run_bass_kernel_spmd(nc, in_maps, core_ids=list(range(8)), trace=trace)
    if trace:
        _CACHE["last_result"] = res
    return np.stack(
        [_unpack_out(res.results[b]["out"]) for b in range(8)]
    ).astype(np.float32)
